# revision 1
# baseline (speedup 1.0000x reference)
"""CfC RNN scan kernel for Trainium2 (8 NeuronCores, data-parallel over batch).

Math (per step, from the reference):
    f   = 1.7159 * tanh(0.666 * (concat(x_s, h) @ W0 + b0))     x_s = (x-65)/100
    ff1 = f @ W1 + b1 ;  ff2 = f @ W2 + b2
    ta  = f @ Wa + ba ;  tb  = f @ Wb + bb
    t   = sigmoid(tb - ta * ts)
    h'  = ff1 + t * (ff2 - ff1)

Folding done on the host:
  - input scale/shift folded into W0x, b0:  xterm = x @ (W0x/100) + (b0 - .65*W0x.sum(0))
  - 1.7159 folded into the head weights; heads consume g = tanh(0.666*z) directly
  - d = ff2-ff1 computed via Wd = W2-W1, bd = b2-b1
  - head weights concatenated: Wcat = [W1' | Wd' | Wa' | Wb'] (256 x 512)

On-chip structure (per core, B_local=32, all fp32):
  - x is fed pre-transposed as xT [C+1, S, BL] (row C = ones so b0 rides the
    matmul); per 8-step chunk one matmul pair computes the x-dependent backbone
    term for all 8 steps straight into PSUM; the recurrent matmul accumulates
    on top (no eviction/preload).
  - All persistent constants (W0aug, W0h, Wcat, h0T, bcat, ones) are packed in
    a single "blob" tensor loaded by ONE DMA: the HW Matmult instruction only
    tolerates a single semaphore wait, so every matmul must depend on at most
    one non-PE producer.  A dummy 1x1x1 warm-up matmul absorbs the blob wait.
  - scan step: hT [128,32] -> MM1 accumulate -> ACT tanh [128,2,32] -> g;
    heads use g as the (P=32) stationary operand: psA=[ta|tb], psB=[ff1|d] in
    separate PSUM banks; per-bank K=1 ones-row matmuls add the biases
    (h-independent, off the critical path).
  - gate: DVE tensor_scalar (ta*-ts, PSUM->SBUF), DVE add (+tb), ACT sigmoid,
    DVE mul (*d), DVE add (+ff1) written into the output staging tile; 4 DVE
    32x32 transposes produce hT for the next step.
"""

import sys

import numpy as np

for _p in ("/opt/trn_rl_repo",):
    if _p not in sys.path:
        sys.path.insert(0, _p)

B, S, C, U, H = 256, 2048, 64, 128, 256
NCORES = 8
BL = B // NCORES  # 32
CHUNK = 32
TS_SUPER = 256  # steps per timespan staging DMA

# blob column layout (128 partitions x BLOB_COLS fp32)
_C_W0AUG = 0          # [65, 256]
_C_W0H = 256          # [128, 256]
_C_WCAT = 512         # [128, 1024] = 2 K-tiles x 512
_C_H0T = 1536         # [128, 32]
_C_BC = 1568          # [1, 512] bcat
_C_ONES = 2080        # [1, 32] ones
BLOB_COLS = 2112


def _build_nc(s_total: int):
    import concourse.bass as bass
    import concourse.tile as tile
    from concourse import mybir
    from concourse.tile_rust import add_dep_helper
    import concourse.tile_sem_assignment as _tsa

    # All DMAs go through gpsimd/SWDGE; cap the SWDGE sem count so the
    # kernel-tail Drain's per-queue waits fit its struct's wait slots.
    _tsa.NUM_SWDGE_GLOBAL_SEMS = 2

    f32 = mybir.dt.float32
    AF = mybir.ActivationFunctionType
    nchunk = s_total // CHUNK
    ts_super = min(TS_SUPER, s_total)

    nc = bass.Bass("TRN2")
    xT_d = nc.dram_tensor("xT", [C + 1, s_total, BL], f32, kind="ExternalInput")
    nts_d = nc.dram_tensor("nts", [BL, s_total], f32, kind="ExternalInput")
    blob_d = nc.dram_tensor("blob", [128, BLOB_COLS], f32, kind="ExternalInput")
    out_d = nc.dram_tensor("out", [BL, s_total, U], f32, kind="ExternalOutput")

    with tile.TileContext(nc) as tc:
        with (
            tc.tile_pool(name="singles", bufs=1) as singles,
            tc.tile_pool(name="xstage", bufs=2) as xstage,
            tc.tile_pool(name="tsstage", bufs=2) as tsstage,
            tc.tile_pool(name="outstage", bufs=2) as outstage,
            tc.tile_pool(name="ft", bufs=6) as ftp,
            tc.tile_pool(name="fb", bufs=6) as fbp,
            tc.tile_pool(name="gate", bufs=6) as gatep,
            tc.tile_pool(name="ht", bufs=2) as htp,
            tc.tile_pool(name="psf", bufs=3, space="PSUM") as psfp,
            tc.tile_pool(name="psbnd", bufs=1, space="PSUM") as psbndp,
            tc.tile_pool(name="psa", bufs=2, space="PSUM") as psap,
            tc.tile_pool(name="psb", bufs=2, space="PSUM") as psbp,
        ):
            sb_blob = singles.tile([128, BLOB_COLS], f32, tag="blob")
            nc.gpsimd.dma_start(out=sb_blob, in_=blob_d[:, :])

            sb_W0aug = sb_blob[0 : C + 1, _C_W0AUG : _C_W0AUG + H]
            sb_W0h = sb_blob[:, _C_W0H : _C_W0H + H]
            sb_h0T = sb_blob[:, _C_H0T : _C_H0T + BL]
            sb_scr = singles.tile([1, 16], f32, tag="scratch")
            # a zero row of the blob: row 64 of the bcat column range (only
            # row 0 holds data there); base partition must be 0/32/64
            sb_zrow = sb_blob[64:65, _C_BC : _C_BC + 256]
            sb_bcat = sb_blob[0:1, _C_BC : _C_BC + 4 * U]
            sb_ones = sb_blob[0:1, _C_ONES : _C_ONES + BL]

            def wcat(k2, lo, hi):
                base = _C_WCAT + k2 * 4 * U
                return sb_blob[:, base + lo : base + hi]

            # warm-up: a 1x1x1 matmul so PE observes the blob DMA's semaphore
            # before any real matmul (Matmult carries at most one sync wait);
            # reuses a psa slot so no extra PSUM bank is consumed
            ps_w = psap.tile([BL, 2 * U], f32, tag="psa")
            nc.tensor.matmul(
                ps_w[0:1, 0:1], sb_blob[0:1, 0:1], sb_blob[0:1, 0:1],
                start=True, stop=True,
            )

            cur_hT = sb_h0T
            prev_pe = None  # last PE instruction of the previous step
            prev_act = None  # nosync chain pinning the ACT instruction order

            for ci in range(nchunk):
                s0 = ci * CHUNK
                xTa = xstage.tile([C + 1, CHUNK * BL], f32, tag="xta")
                nc.gpsimd.dma_start(out=xTa, in_=xT_d[:, s0 : s0 + CHUNK, :])
                if s0 % ts_super == 0:
                    ntss = tsstage.tile([BL, ts_super], f32, tag="ntss")
                    nc.gpsimd.dma_start(out=ntss, in_=nts_d[:, s0 : s0 + ts_super])
                    # DVE toucher: absorb the ntss DMA wait on DVE once, so
                    # per-step tensor_scalar ops don't carry a second wait
                    sci = s0 // ts_super
                    nc.vector.tensor_copy(
                        sb_scr[0:1, sci : sci + 1], ntss[0:1, 0:1]
                    )

                ostage = outstage.tile([BL, CHUNK * U], f32, tag="ostage")
                # DVE toucher: absorb the WAR on the previous out-DMA of this
                # staging buffer so the first nh write has only one wait
                nc.vector.memset(ostage[0:1, 0:1], 0.0)

                for s in range(CHUNK):
                    st = (s0 + s) % ts_super  # index into ntss
                    # backbone: z = x-term + W0h.T @ hT, one accumulation group
                    # per m-tile (the x-term matmul is h-independent and runs
                    # ahead; same-group accumulation avoids extra PE waits)
                    # chunk-boundary step uses a dedicated psum tile: its
                    # slot-reuse WAW wait is then chunk-distant (dominated),
                    # leaving room for the xTa DMA wait (1-wait limit)
                    if s == 0:
                        ps_f = psbndp.tile([128, 2, BL], f32, tag="psbnd")
                    else:
                        ps_f = psfp.tile([128, 2, BL], f32, tag="psf")
                    # start=True clears the ENTIRE psum bank, so the two
                    # m-tiles (sharing one bank) must not each lead their own
                    # group: one K=1 zero-matmul clears/claims the whole
                    # region, everything else accumulates.
                    clr = nc.tensor.matmul(
                        ps_f,
                        sb_zrow[:, 0:128],
                        sb_zrow[:, 0 : 2 * BL],
                        start=True,
                        stop=False,
                        skip_group_check=True,
                    )
                    if prev_pe is not None:
                        add_dep_helper(clr.ins, prev_pe.ins, False, "clr after heads")
                    for m in range(2):
                        nc.tensor.matmul(
                            ps_f[:, m, :],
                            sb_W0aug[:, m * 128 : (m + 1) * 128],
                            xTa[:, s * BL : (s + 1) * BL],
                            start=False,
                            stop=False,
                            skip_group_check=True,
                        )
                    mm1_last = None
                    for m in range(2):
                        mm1_last = nc.tensor.matmul(
                            ps_f[:, m, :],
                            sb_W0h[:, m * 128 : (m + 1) * 128],
                            cur_hT,
                            start=False,
                            stop=True,
                            skip_group_check=True,
                        )
                    # g = tanh(0.666 * z), both H-tiles in one ACT op
                    fT = ftp.tile([128, 2, BL], f32, tag="ft")
                    th = nc.scalar.activation(fT, ps_f, AF.Tanh, scale=0.666)
                    if prev_act is not None:
                        # nosync chain: fixes the ACT stream order so slot
                        # reuse stays outside the queue window and no ACT
                        # self-waits are emitted (Activation has 1 wait slot)
                        add_dep_helper(th.ins, prev_act.ins, False, "act chain")
                    prev_act = th

                    # heads: psA = [ta | tb], psB = [ff1 | d] (separate banks)
                    psA = psap.tile([BL, 2 * U], f32, tag="psa")
                    psB = psbp.tile([BL, 2 * U], f32, tag="psb")
                    # order-only dep: keep the bias matmuls behind this
                    # step's MM1 so their psum-WAR wait is dominated by MM1's
                    # DVE wait (Matmult tolerates only one sync wait)
                    bmA = nc.tensor.matmul(
                        psA, sb_ones, sb_bcat[:, 2 * U : 4 * U], start=True, stop=False
                    )
                    bmB = nc.tensor.matmul(
                        psB, sb_ones, sb_bcat[:, 0 : 2 * U], start=True, stop=False
                    )
                    add_dep_helper(bmA.ins, mm1_last.ins, False, "bias after MM1")
                    add_dep_helper(bmB.ins, mm1_last.ins, False, "bias after MM1")
                    for k2 in range(2):
                        nc.tensor.matmul(
                            psA,
                            fT[:, k2, :],
                            wcat(k2, 2 * U, 4 * U),
                            start=False,
                            stop=(k2 == 1),
                        )
                    for k2 in range(2):
                        prev_pe = nc.tensor.matmul(
                            psB,
                            fT[:, k2, :],
                            wcat(k2, 0, 2 * U),
                            start=False,
                            stop=(k2 == 1),
                        )

                    # gate: v = tb - ta*ts ; t = sigmoid(v) ; nh = ff1 + t*d
                    # (only one PSUM input allowed per DVE op). psB is evicted
                    # to SBUF on ACT (hidden behind t1/v) so t3's single ACT
                    # wait covers both the sigmoid and [ff1|d].
                    t1 = gatep.tile([BL, U], f32, tag="t1")
                    nc.vector.tensor_scalar_mul(t1, psA[:, 0:U], ntss[:, st : st + 1])
                    v = gatep.tile([BL, U], f32, tag="v")
                    nc.vector.tensor_add(v, t1, psA[:, U : 2 * U])
                    fB = fbp.tile([BL, 2 * U], f32, tag="fb")
                    cb = nc.scalar.copy(fB, psB)
                    add_dep_helper(cb.ins, prev_act.ins, False, "act chain")
                    prev_act = cb
                    sg = gatep.tile([BL, U], f32, tag="sg")
                    sgi = nc.scalar.activation(sg, v, AF.Sigmoid)
                    add_dep_helper(sgi.ins, prev_act.ins, False, "act chain")
                    prev_act = sgi
                    t3 = gatep.tile([BL, U], f32, tag="t3")
                    nc.vector.tensor_mul(t3, sg, fB[:, U : 2 * U])
                    nh = ostage[:, s * U : (s + 1) * U]
                    nc.vector.tensor_add(nh, t3, fB[:, 0:U])

                    # hT for the next step: 4x 32x32 DVE transposes
                    hT = htp.tile([U, BL], f32, tag="ht")
                    for j in range(4):
                        nc.vector.transpose(
                            hT[32 * j : 32 * (j + 1), :],
                            nh[:, 32 * j : 32 * (j + 1)],
                        )
                    cur_hT = hT

                nc.gpsimd.dma_start(out=out_d[:, s0 : s0 + CHUNK, :], in_=ostage)

    _drop_stale_self_waits(nc, mybir)
    return nc


def _drop_stale_self_waits(nc, mybir, margin=8):
    """Compute instructions have a single usable wait slot (the engine-sem
    update takes the other).  Tile emits same-engine/same-lane waits for
    slot reuse even when the producer is far back; on an in-order engine or
    FIFO DMA lane those are redundant.  Drop self waits on instructions
    carrying >1 wait: engine-sem waits when >= `margin` instructions stale,
    own-DMA-lane waits always (the lane is FIFO)."""
    eng_prefix = {
        mybir.EngineType.PE: "PE",
        mybir.EngineType.DVE: "DVE",
        mybir.EngineType.Activation: "Activation",
        mybir.EngineType.Pool: "Pool",
        mybir.EngineType.SP: "SP",
    }
    tick = {}
    for fn in nc.m.functions:
        for blk in fn.blocks:
            for i in blk.instructions:
                si = i.sync_info
                if si is None:
                    continue
                pfx = eng_prefix.get(getattr(i, "engine", None))
                upd_sems = {u.ant_name for u in si.on_update}
                if len(si.on_wait) > 1:
                    is_dma = type(i).__name__ == "InstDMACopy"
                    kept = []
                    for w in si.on_wait:
                        n = w.ant_name
                        if (
                            pfx
                            and n.startswith(pfx + "_")
                            and tick.get(n, 0) - w.wait_value >= margin
                        ):
                            continue  # stale engine self-wait
                        if (
                            is_dma
                            and n in upd_sems
                            and ("DMASW" in n or "DMAHW" in n)
                            and tick.get(n, 0) >= w.wait_value
                        ):
                            continue  # own-lane FIFO wait
                        kept.append(w)
                    if len(kept) != len(si.on_wait):
                        si.on_wait = kept
                for u in si.on_update:
                    tick[u.ant_name] = tick.get(u.ant_name, 0) + u.update_value
    _split_multiwait_drains(nc, mybir)


def _split_multiwait_drains(nc, mybir):
    """The kernel-tail Drain waits on every engine/DMA-lane sem, but its
    struct has a single wait slot.  Split: inject one single-wait Drain per
    extra wait immediately before it on the same engine."""
    for fn in nc.m.functions:
        for blk in fn.blocks:
            insts = blk.instructions
            out = []
            changed = False
            for i in insts:
                si = i.sync_info
                if type(i).__name__ == "InstDrain" and si and len(si.on_wait) > 1:
                    waits = list(si.on_wait)
                    for k, w in enumerate(waits[:-1]):
                        d = mybir.InstDrain(name=f"{i.name}-w{k}", ins=[], outs=[])
                        d.engine = i.engine
                        d.sync_info = mybir.SyncInfo(on_wait=[w], on_update=[])
                        out.append(d)
                    si.on_wait = [waits[-1]]
                    changed = True
                out.append(i)
            if changed:
                blk.instructions = out


def _prep_weights(W0, b0, W1, b1, W2, b2, Wa, ba, Wb, bb):
    W0 = np.asarray(W0, np.float32)
    W0x = W0[:C] / 100.0
    W0h = np.ascontiguousarray(W0[C:])  # [U, H]
    b0p = np.asarray(b0, np.float32) - 0.65 * W0[:C].sum(axis=0)
    W0aug = np.concatenate([W0x, b0p[None, :]], axis=0)  # [C+1, H]
    a = np.float32(1.7159)
    Wcat = np.concatenate([a * W1, a * (W2 - W1), a * Wa, a * Wb], axis=1)  # [H, 4U]
    bcat = np.concatenate([b1, b2 - b1, ba, bb]).astype(np.float32)  # [4U]
    return (
        W0aug.astype(np.float32),
        W0h.astype(np.float32),
        Wcat.astype(np.float32),
        bcat,
    )


def _make_blob(weights, h0T):
    W0aug, W0h, Wcat, bcat = weights
    blob = np.zeros((128, BLOB_COLS), np.float32)
    blob[0 : C + 1, _C_W0AUG : _C_W0AUG + H] = W0aug
    blob[:, _C_W0H : _C_W0H + H] = W0h
    for k2 in range(2):
        blob[:, _C_WCAT + k2 * 4 * U : _C_WCAT + (k2 + 1) * 4 * U] = Wcat[
            k2 * 128 : (k2 + 1) * 128, :
        ]
    blob[:, _C_H0T : _C_H0T + BL] = h0T
    blob[0, _C_BC : _C_BC + 4 * U] = bcat
    blob[0, _C_ONES : _C_ONES + BL] = 1.0
    return blob


def _make_in_maps(x_codes, h0, timespans, weights, s_total):
    in_maps = []
    for i in range(NCORES):
        lo, hi = i * BL, (i + 1) * BL
        xb = np.empty((C + 1, s_total, BL), np.float32)
        xb[:C] = np.transpose(x_codes[lo:hi, :s_total, :], (2, 1, 0))
        xb[C] = 1.0  # ones plane: carries b0 through the phase-A matmul
        nts = np.ascontiguousarray(-timespans[lo:hi, :s_total], np.float32)
        h0T = np.ascontiguousarray(h0[lo:hi].T, np.float32)  # [U, BL]
        in_maps.append({"xT": xb, "nts": nts, "blob": _make_blob(weights, h0T)})
    return in_maps


_CACHE = {}


def run(x_codes, h0, timespans, weights, s_total=S, trace=False):
    from concourse.bass_utils import run_bass_kernel_spmd

    key = s_total
    if key not in _CACHE:
        _CACHE[key] = _build_nc(s_total)
    nc = _CACHE[key]
    in_maps = _make_in_maps(x_codes, h0, timespans, weights, s_total)
    res = run_bass_kernel_spmd(nc, in_maps, core_ids=list(range(NCORES)), trace=trace)
    outs = [r["out"] for r in res.results]
    full = np.concatenate(outs, axis=0)  # [B, S, U]
    return full, res


def kernel(x_codes, h0, timespans, W0, b0, W1, b1, W2, b2, Wa, ba, Wb, bb):
    weights = _prep_weights(W0, b0, W1, b1, W2, b2, Wa, ba, Wb, bb)
    full, _ = run(
        np.asarray(x_codes, np.float32),
        np.asarray(h0, np.float32),
        np.asarray(timespans, np.float32),
        weights,
        S,
    )
    return full.astype(np.float32)



# revision 3
# speedup vs baseline: 1.1677x; 1.1677x over previous
"""CfC RNN scan kernel for Trainium2 (8 NeuronCores, data-parallel over batch).

Math (per step, from the reference):
    f   = 1.7159 * tanh(0.666 * (concat(x_s, h) @ W0 + b0))     x_s = (x-65)/100
    ff1 = f @ W1 + b1 ;  ff2 = f @ W2 + b2
    ta  = f @ Wa + ba ;  tb  = f @ Wb + bb
    t   = sigmoid(tb - ta * ts)
    h'  = ff1 + t * (ff2 - ff1)

Folding done on the host:
  - input scale/shift folded into W0x, b0:  xterm = x @ (W0x/100) + (b0 - .65*W0x.sum(0))
  - 1.7159 folded into the head weights; heads consume g = tanh(0.666*z) directly
  - d = ff2-ff1 computed via Wd = W2-W1, bd = b2-b1
  - head weights concatenated: Wcat = [W1' | Wd' | Wa' | Wb'] (256 x 512)

On-chip structure (per core, B_local=32, all fp32):
  - x is fed pre-transposed as xT [C+1, S, BL] (row C = ones so b0 rides the
    matmul); per 8-step chunk one matmul pair computes the x-dependent backbone
    term for all 8 steps straight into PSUM; the recurrent matmul accumulates
    on top (no eviction/preload).
  - All persistent constants (W0aug, W0h, Wcat, h0T, bcat, ones) are packed in
    a single "blob" tensor loaded by ONE DMA: the HW Matmult instruction only
    tolerates a single semaphore wait, so every matmul must depend on at most
    one non-PE producer.  A dummy 1x1x1 warm-up matmul absorbs the blob wait.
  - scan step: hT [128,32] -> MM1 accumulate -> ACT tanh [128,2,32] -> g;
    heads use g as the (P=32) stationary operand: psA=[ta|tb], psB=[ff1|d] in
    separate PSUM banks; per-bank K=1 ones-row matmuls add the biases
    (h-independent, off the critical path).
  - gate: DVE tensor_scalar (ta*-ts, PSUM->SBUF), DVE add (+tb), ACT sigmoid,
    DVE mul (*d), DVE add (+ff1) written into the output staging tile; 4 DVE
    32x32 transposes produce hT for the next step.
"""

import sys

import numpy as np

for _p in ("/opt/trn_rl_repo",):
    if _p not in sys.path:
        sys.path.insert(0, _p)

B, S, C, U, H = 256, 2048, 64, 128, 256
NCORES = 8
BL = B // NCORES  # 32
CHUNK = 32
TS_SUPER = 256  # steps per timespan staging DMA

# blob column layout (128 partitions x BLOB_COLS fp32)
_C_W0AUG = 0          # [65, 256]
_C_W0H = 256          # [128, 256]
_C_WCAT = 512         # [128, 1024] = 2 K-tiles x 512
_C_H0T = 1536         # [128, 32]
_C_BC = 1568          # [1, 512] bcat
_C_ONES = 2080        # [1, 32] ones
BLOB_COLS = 2112


def _build_nc(s_total: int):
    import concourse.bass as bass
    import concourse.tile as tile
    from concourse import mybir
    from concourse.tile_rust import add_dep_helper
    import concourse.tile_sem_assignment as _tsa

    # All DMAs go through gpsimd/SWDGE; cap the SWDGE sem count so the
    # kernel-tail Drain's per-queue waits fit its struct's wait slots.
    _tsa.NUM_SWDGE_GLOBAL_SEMS = 2

    f32 = mybir.dt.float32
    AF = mybir.ActivationFunctionType
    nchunk = s_total // CHUNK
    ts_super = min(TS_SUPER, s_total)

    nc = bass.Bass("TRN2")
    xT_d = nc.dram_tensor("xT", [C + 1, s_total, BL], f32, kind="ExternalInput")
    nts_d = nc.dram_tensor("nts", [BL, s_total], f32, kind="ExternalInput")
    blob_d = nc.dram_tensor("blob", [128, BLOB_COLS], f32, kind="ExternalInput")
    out_d = nc.dram_tensor("out", [BL, s_total, U], f32, kind="ExternalOutput")

    with tile.TileContext(nc) as tc:
        with (
            tc.tile_pool(name="singles", bufs=1) as singles,
            tc.tile_pool(name="xstage", bufs=2) as xstage,
            tc.tile_pool(name="tsstage", bufs=2) as tsstage,
            tc.tile_pool(name="outstage", bufs=2) as outstage,
            tc.tile_pool(name="ft", bufs=6) as ftp,
            tc.tile_pool(name="fb", bufs=6) as fbp,
            tc.tile_pool(name="gate", bufs=6) as gatep,
            tc.tile_pool(name="ht", bufs=2) as htp,
            tc.tile_pool(name="psf", bufs=3, space="PSUM") as psfp,
            tc.tile_pool(name="psbnd", bufs=1, space="PSUM") as psbndp,
            tc.tile_pool(name="psa", bufs=2, space="PSUM") as psap,
            tc.tile_pool(name="psb", bufs=2, space="PSUM") as psbp,
        ):
            sb_blob = singles.tile([128, BLOB_COLS], f32, tag="blob")
            nc.gpsimd.dma_start(out=sb_blob, in_=blob_d[:, :])

            sb_W0aug = sb_blob[0 : C + 1, _C_W0AUG : _C_W0AUG + H]
            sb_W0h = sb_blob[:, _C_W0H : _C_W0H + H]
            sb_h0T = sb_blob[:, _C_H0T : _C_H0T + BL]
            sb_scr = singles.tile([1, 16], f32, tag="scratch")
            # a zero row of the blob: row 64 of the bcat column range (only
            # row 0 holds data there); base partition must be 0/32/64
            sb_zrow = sb_blob[64:65, _C_BC : _C_BC + 256]
            sb_bcat = sb_blob[0:1, _C_BC : _C_BC + 4 * U]
            sb_ones = sb_blob[0:1, _C_ONES : _C_ONES + BL]

            def wcat(k2, lo, hi):
                base = _C_WCAT + k2 * 4 * U
                return sb_blob[:, base + lo : base + hi]

            # warm-up: a 1x1x1 matmul so PE observes the blob DMA's semaphore
            # before any real matmul (Matmult carries at most one sync wait);
            # reuses a psa slot so no extra PSUM bank is consumed
            ps_w = psap.tile([BL, 2 * U], f32, tag="psa")
            nc.tensor.matmul(
                ps_w[0:1, 0:1], sb_blob[0:1, 0:1], sb_blob[0:1, 0:1],
                start=True, stop=True,
            )

            cur_hT = sb_h0T
            prev_pe = None  # last PE instruction of the previous step
            prev_act = None  # nosync chain pinning the ACT instruction order

            for ci in range(nchunk):
                s0 = ci * CHUNK
                xTa = xstage.tile([C + 1, CHUNK * BL], f32, tag="xta")
                nc.gpsimd.dma_start(out=xTa, in_=xT_d[:, s0 : s0 + CHUNK, :])
                if s0 % ts_super == 0:
                    ntss = tsstage.tile([BL, ts_super], f32, tag="ntss")
                    nc.gpsimd.dma_start(out=ntss, in_=nts_d[:, s0 : s0 + ts_super])
                    # DVE toucher: absorb the ntss DMA wait on DVE once, so
                    # per-step tensor_scalar ops don't carry a second wait
                    sci = s0 // ts_super
                    nc.vector.tensor_copy(
                        sb_scr[0:1, sci : sci + 1], ntss[0:1, 0:1]
                    )

                ostage = outstage.tile([BL, CHUNK * U], f32, tag="ostage")
                # DVE toucher: absorb the WAR on the previous out-DMA of this
                # staging buffer so the first nh write has only one wait
                nc.vector.memset(ostage[0:1, 0:1], 0.0)

                for s in range(CHUNK):
                    st = (s0 + s) % ts_super  # index into ntss
                    # backbone: z = x-term + W0h.T @ hT, one accumulation group
                    # per m-tile (the x-term matmul is h-independent and runs
                    # ahead; same-group accumulation avoids extra PE waits)
                    # chunk-boundary step uses a dedicated psum tile: its
                    # slot-reuse WAW wait is then chunk-distant (dominated),
                    # leaving room for the xTa DMA wait (1-wait limit)
                    if s == 0:
                        ps_f = psbndp.tile([128, 2, BL], f32, tag="psbnd")
                    else:
                        ps_f = psfp.tile([128, 2, BL], f32, tag="psf")
                    # start=True clears the ENTIRE psum bank, so the two
                    # m-tiles (sharing one bank) must not each lead their own
                    # group: one K=1 zero-matmul clears/claims the whole
                    # region, everything else accumulates.
                    clr = nc.tensor.matmul(
                        ps_f,
                        sb_zrow[:, 0:128],
                        sb_zrow[:, 0 : 2 * BL],
                        start=True,
                        stop=False,
                        skip_group_check=True,
                    )
                    if prev_pe is not None:
                        add_dep_helper(clr.ins, prev_pe.ins, False, "clr after heads")
                    for m in range(2):
                        nc.tensor.matmul(
                            ps_f[:, m, :],
                            sb_W0aug[:, m * 128 : (m + 1) * 128],
                            xTa[:, s * BL : (s + 1) * BL],
                            start=False,
                            stop=False,
                            skip_group_check=True,
                        )
                    mm1_last = None
                    for m in range(2):
                        mm1_last = nc.tensor.matmul(
                            ps_f[:, m, :],
                            sb_W0h[:, m * 128 : (m + 1) * 128],
                            cur_hT,
                            start=False,
                            stop=True,
                            skip_group_check=True,
                        )
                    # g = tanh(0.666 * z), both H-tiles in one ACT op
                    fT = ftp.tile([128, 2, BL], f32, tag="ft")
                    th = nc.scalar.activation(fT, ps_f, AF.Tanh, scale=0.666)
                    if prev_act is not None:
                        # nosync chain: fixes the ACT stream order so slot
                        # reuse stays outside the queue window and no ACT
                        # self-waits are emitted (Activation has 1 wait slot)
                        add_dep_helper(th.ins, prev_act.ins, False, "act chain")
                    prev_act = th

                    # heads: psA = [ta | tb], psB = [ff1 | d] (separate banks)
                    psA = psap.tile([BL, 2 * U], f32, tag="psa")
                    psB = psbp.tile([BL, 2 * U], f32, tag="psb")
                    # order-only dep: keep the bias matmuls behind this
                    # step's MM1 so their psum-WAR wait is dominated by MM1's
                    # DVE wait (Matmult tolerates only one sync wait)
                    bmA = nc.tensor.matmul(
                        psA, sb_ones, sb_bcat[:, 2 * U : 4 * U], start=True, stop=False
                    )
                    bmB = nc.tensor.matmul(
                        psB, sb_ones, sb_bcat[:, 0 : 2 * U], start=True, stop=False
                    )
                    add_dep_helper(bmA.ins, mm1_last.ins, False, "bias after MM1")
                    add_dep_helper(bmB.ins, mm1_last.ins, False, "bias after MM1")
                    for k2 in range(2):
                        nc.tensor.matmul(
                            psA,
                            fT[:, k2, :],
                            wcat(k2, 2 * U, 4 * U),
                            start=False,
                            stop=(k2 == 1),
                        )
                    for k2 in range(2):
                        prev_pe = nc.tensor.matmul(
                            psB,
                            fT[:, k2, :],
                            wcat(k2, 0, 2 * U),
                            start=False,
                            stop=(k2 == 1),
                        )

                    # gate: v = tb - ta*ts ; t = sigmoid(v) ; nh = ff1 + t*d
                    # (only one PSUM input allowed per DVE op). psB is evicted
                    # to SBUF on ACT (hidden behind t1/v) so t3's single ACT
                    # wait covers both the sigmoid and [ff1|d].
                    t1 = gatep.tile([BL, U], f32, tag="t1")
                    nc.vector.tensor_scalar_mul(t1, psA[:, 0:U], ntss[:, st : st + 1])
                    v = gatep.tile([BL, U], f32, tag="v")
                    nc.vector.tensor_add(v, t1, psA[:, U : 2 * U])
                    fB = fbp.tile([BL, 2 * U], f32, tag="fb")
                    cb = nc.scalar.copy(fB, psB)
                    add_dep_helper(cb.ins, prev_act.ins, False, "act chain")
                    prev_act = cb
                    sg = gatep.tile([BL, U], f32, tag="sg")
                    sgi = nc.scalar.activation(sg, v, AF.Sigmoid)
                    add_dep_helper(sgi.ins, prev_act.ins, False, "act chain")
                    prev_act = sgi
                    t3 = gatep.tile([BL, U], f32, tag="t3")
                    nc.vector.tensor_mul(t3, sg, fB[:, U : 2 * U])
                    nh = ostage[:, s * U : (s + 1) * U]
                    nc.vector.tensor_add(nh, t3, fB[:, 0:U])

                    # hT for the next step: 4x 32x32 DVE transposes
                    hT = htp.tile([U, BL], f32, tag="ht")
                    for j in range(4):
                        nc.vector.transpose(
                            hT[32 * j : 32 * (j + 1), :],
                            nh[:, 32 * j : 32 * (j + 1)],
                        )
                    cur_hT = hT

                nc.gpsimd.dma_start(out=out_d[:, s0 : s0 + CHUNK, :], in_=ostage)

    _drop_stale_self_waits(nc, mybir)
    return nc


def _drop_stale_self_waits(nc, mybir, margin=8):
    """Compute instructions have a single usable wait slot (the engine-sem
    update takes the other).  Tile emits same-engine/same-lane waits for
    slot reuse even when the producer is far back; on an in-order engine or
    FIFO DMA lane those are redundant.  Drop self waits on instructions
    carrying >1 wait: engine-sem waits when >= `margin` instructions stale,
    own-DMA-lane waits always (the lane is FIFO)."""
    eng_prefix = {
        mybir.EngineType.PE: "PE",
        mybir.EngineType.DVE: "DVE",
        mybir.EngineType.Activation: "Activation",
        mybir.EngineType.Pool: "Pool",
        mybir.EngineType.SP: "SP",
    }
    tick = {}
    for fn in nc.m.functions:
        for blk in fn.blocks:
            for i in blk.instructions:
                si = i.sync_info
                if si is None:
                    continue
                pfx = eng_prefix.get(getattr(i, "engine", None))
                upd_sems = {u.ant_name for u in si.on_update}
                if len(si.on_wait) > 1:
                    is_dma = type(i).__name__ == "InstDMACopy"
                    kept = []
                    for w in si.on_wait:
                        n = w.ant_name
                        if (
                            pfx
                            and n.startswith(pfx + "_")
                            and tick.get(n, 0) - w.wait_value >= margin
                        ):
                            continue  # stale engine self-wait
                        if (
                            is_dma
                            and n in upd_sems
                            and ("DMASW" in n or "DMAHW" in n)
                            and tick.get(n, 0) >= w.wait_value
                        ):
                            continue  # own-lane FIFO wait
                        kept.append(w)
                    if len(kept) != len(si.on_wait):
                        si.on_wait = kept
                for u in si.on_update:
                    tick[u.ant_name] = tick.get(u.ant_name, 0) + u.update_value
    _split_multiwait_drains(nc, mybir)


def _split_multiwait_drains(nc, mybir):
    """The kernel-tail Drain waits on every engine/DMA-lane sem, but its
    struct has a single wait slot.  Split: inject one single-wait Drain per
    extra wait immediately before it on the same engine."""
    for fn in nc.m.functions:
        for blk in fn.blocks:
            insts = blk.instructions
            out = []
            changed = False
            for i in insts:
                si = i.sync_info
                if type(i).__name__ == "InstDrain" and si and len(si.on_wait) > 1:
                    waits = list(si.on_wait)
                    for k, w in enumerate(waits[:-1]):
                        d = mybir.InstDrain(name=f"{i.name}-w{k}", ins=[], outs=[])
                        d.engine = i.engine
                        d.sync_info = mybir.SyncInfo(on_wait=[w], on_update=[])
                        out.append(d)
                    si.on_wait = [waits[-1]]
                    changed = True
                out.append(i)
            if changed:
                blk.instructions = out


def _prep_weights(W0, b0, W1, b1, W2, b2, Wa, ba, Wb, bb):
    W0 = np.asarray(W0, np.float32)
    W0x = W0[:C] / 100.0
    W0h = np.ascontiguousarray(W0[C:])  # [U, H]
    b0p = np.asarray(b0, np.float32) - 0.65 * W0[:C].sum(axis=0)
    W0aug = np.concatenate([W0x, b0p[None, :]], axis=0)  # [C+1, H]
    a = np.float32(1.7159)
    Wcat = np.concatenate([a * W1, a * (W2 - W1), a * Wa, a * Wb], axis=1)  # [H, 4U]
    bcat = np.concatenate([b1, b2 - b1, ba, bb]).astype(np.float32)  # [4U]
    return (
        W0aug.astype(np.float32),
        W0h.astype(np.float32),
        Wcat.astype(np.float32),
        bcat,
    )


def _make_blob(weights, h0T):
    W0aug, W0h, Wcat, bcat = weights
    blob = np.zeros((128, BLOB_COLS), np.float32)
    blob[0 : C + 1, _C_W0AUG : _C_W0AUG + H] = W0aug
    blob[:, _C_W0H : _C_W0H + H] = W0h
    for k2 in range(2):
        blob[:, _C_WCAT + k2 * 4 * U : _C_WCAT + (k2 + 1) * 4 * U] = Wcat[
            k2 * 128 : (k2 + 1) * 128, :
        ]
    blob[:, _C_H0T : _C_H0T + BL] = h0T
    blob[0, _C_BC : _C_BC + 4 * U] = bcat
    blob[0, _C_ONES : _C_ONES + BL] = 1.0
    return blob


def _make_in_maps(x_codes, h0, timespans, weights, s_total):
    in_maps = []
    for i in range(NCORES):
        lo, hi = i * BL, (i + 1) * BL
        xb = np.empty((C + 1, s_total, BL), np.float32)
        xb[:C] = np.transpose(x_codes[lo:hi, :s_total, :], (2, 1, 0))
        xb[C] = 1.0  # ones plane: carries b0 through the phase-A matmul
        nts = np.ascontiguousarray(-timespans[lo:hi, :s_total], np.float32)
        h0T = np.ascontiguousarray(h0[lo:hi].T, np.float32)  # [U, BL]
        in_maps.append({"xT": xb, "nts": nts, "blob": _make_blob(weights, h0T)})
    return in_maps


_CACHE = {}
_EXEC_CACHE = {}


def _ensure_exec(s_total: int):
    """Build the Bass module once and AOT-compile the sharded PJRT callable;
    cache both so repeat calls skip trace/lower/compile/load entirely.
    Mirrors run_bass_via_pjrt exactly (same in_names order, specs, donation)
    so the lowered HLO -- and therefore the terminal NEFF cache key -- is
    identical to the run_bass_kernel_spmd path."""
    if s_total in _EXEC_CACHE:
        return _EXEC_CACHE[s_total]

    import jax
    from jax.experimental.shard_map import shard_map
    from jax.sharding import Mesh, PartitionSpec

    from concourse import mybir
    from concourse.bass2jax import (
        _bass_exec_p,
        install_neuronx_cc_hook,
        partition_id_tensor,
    )

    install_neuronx_cc_hook()
    if s_total not in _CACHE:
        _CACHE[s_total] = _build_nc(s_total)
    nc = _CACHE[s_total]

    partition_name = nc.partition_id_tensor.name if nc.partition_id_tensor else None
    in_names = []
    in_shapes = []
    out_names = []
    out_avals = []
    out_shapes = []
    for alloc in nc.m.functions[0].allocations:
        if not isinstance(alloc, mybir.MemoryLocationSet):
            continue
        name = alloc.memorylocations[0].name
        if alloc.kind == "ExternalInput":
            if name != partition_name:
                in_names.append(name)
                in_shapes.append(
                    (tuple(alloc.tensor_shape), mybir.dt.np(alloc.dtype))
                )
        elif alloc.kind == "ExternalOutput":
            shape = tuple(alloc.tensor_shape)
            dtype = mybir.dt.np(alloc.dtype)
            out_avals.append(jax.core.ShapedArray(shape, dtype))
            out_shapes.append((shape, dtype))
            out_names.append(name)
    n_params = len(in_names)
    n_outs = len(out_avals)
    in_names_all = list(in_names) + out_names
    if partition_name is not None:
        in_names_all.append(partition_name)
    donate = tuple(range(n_params, n_params + n_outs))

    def _body(*args):
        operands = list(args)
        if partition_name is not None:
            operands.append(partition_id_tensor())
        outs = _bass_exec_p.bind(
            *operands,
            out_avals=tuple(out_avals),
            in_names=tuple(in_names_all),
            out_names=tuple(out_names),
            lowering_input_output_aliases=(),
            sim_require_finite=True,
            sim_require_nnan=True,
            nc=nc,
        )
        return tuple(outs)

    devices = jax.devices()[:NCORES]
    mesh = Mesh(np.asarray(devices), ("core",))
    in_specs = (PartitionSpec("core"),) * (n_params + n_outs)
    out_specs = (PartitionSpec("core"),) * n_outs
    sharded = jax.jit(
        shard_map(
            _body, mesh=mesh, in_specs=in_specs, out_specs=out_specs, check_rep=False
        ),
        donate_argnums=donate,
        keep_unused=True,
    )
    gin_avals = [
        jax.ShapeDtypeStruct((NCORES * s[0][0],) + tuple(s[0][1:]), s[1])
        for s in in_shapes
    ]
    gz_avals = [
        jax.ShapeDtypeStruct((NCORES * s[0][0],) + tuple(s[0][1:]), s[1])
        for s in out_shapes
    ]
    compiled = sharded.lower(*gin_avals, *gz_avals).compile()
    entry = {
        "compiled": compiled,
        "in_names": in_names,
        "out_shapes": out_shapes,
    }
    _EXEC_CACHE[s_total] = entry
    return entry


def run(x_codes, h0, timespans, weights, s_total=S, trace=False):
    if trace:
        from concourse.bass_utils import run_bass_kernel_spmd

        if s_total not in _CACHE:
            _CACHE[s_total] = _build_nc(s_total)
        nc = _CACHE[s_total]
        in_maps = _make_in_maps(x_codes, h0, timespans, weights, s_total)
        res = run_bass_kernel_spmd(
            nc, in_maps, core_ids=list(range(NCORES)), trace=trace
        )
        outs = [r["out"] for r in res.results]
        full = np.concatenate(outs, axis=0)  # [B, S, U]
        return full, res

    entry = _ensure_exec(s_total)
    in_maps = _make_in_maps(x_codes, h0, timespans, weights, s_total)
    ins = [
        np.concatenate([m[nm] for m in in_maps], axis=0)
        for nm in entry["in_names"]
    ]
    zeros = [
        np.zeros((NCORES * s[0][0],) + tuple(s[0][1:]), s[1])
        for s in entry["out_shapes"]
    ]
    outs = entry["compiled"](*ins, *zeros)
    # Fetch the 8 output shards in parallel threads (the sequential
    # np.asarray path drains them one by one through the relay), then
    # assemble -- shard order along axis 0 is core==batch order.
    from concurrent.futures import ThreadPoolExecutor

    arr = outs[0]
    try:
        shards = sorted(
            arr.addressable_shards, key=lambda sh: sh.index[0].start or 0
        )
        parts = [None] * len(shards)

        def _fetch(i):
            parts[i] = np.asarray(shards[i].data)

        with ThreadPoolExecutor(max_workers=len(shards)) as ex:
            list(ex.map(_fetch, range(len(shards))))
        full = np.concatenate(parts, axis=0)  # [B, S, U]
    except Exception:
        full = np.asarray(arr)
    return full, None


def kernel(x_codes, h0, timespans, W0, b0, W1, b1, W2, b2, Wa, ba, Wb, bb):
    weights = _prep_weights(W0, b0, W1, b1, W2, b2, Wa, ba, Wb, bb)
    full, _ = run(
        np.asarray(x_codes, np.float32),
        np.asarray(h0, np.float32),
        np.asarray(timespans, np.float32),
        weights,
        S,
    )
    return full.astype(np.float32)

